# revision 8
# baseline (speedup 1.0000x reference)
"""Trainium2 Bass kernel for CenterOfMass2DExtractor.

Full input x: (8, 4, 256, 256, 64) float32.  Output: (8, 4, 64) complex64
  mass[b,f,z]   = sum_{i,j} x[b,f,i,j,z]
  real[b,f,z]   = sum_{i,j} j * x / mass      (j = column index)
  imag[b,f,z]   = sum_{i,j} i * x / mass      (i = row index)

Sharding: pure data parallel over the batch dim -> 1 batch per NeuronCore
(8 cores), 64 MiB each, no communication.

Per-core kernel: view the shard as (f=4, t=128, p=128, v=256) where
t indexes blocks of 512 pixels (2 image rows), partition p holds 4
consecutive pixels q=0..3 (v = q*64 + z).  For each t: one 512 KiB DMA
(all 4 f), then 4 matmuls (one per q) with a 3-column stationary weight
  w[p, :] = [1, (4p+q) % 256, 2t + (p >= 64)]  =  [1, j, i]
and moving operand (p, f, z) = 256 columns in float32r (full-rate fp32),
accumulating [mass, sum j*x, sum i*x] into a single (3, 4, 64) PSUM tile
across all 512 matmuls.  The tiny (3, 256) result is copied to SBUF and
DMA'd out; the divide by mass and complex assembly happen on host.
"""

import numpy as np

_CACHE: dict = {}

NB, NF, NX, NY, NZ = 8, 4, 256, 256, 64
PX = 8            # pixels per partition per t-block
NT = 512 // PX    # t-blocks per f (128*PX pixels each)
NP = 128          # partitions
NV = PX * NZ      # values per partition per t-block


def _weights() -> np.ndarray:
    """(p, t, q, c) weight table: c = [mass, j, i]."""
    p = np.arange(NP).reshape(NP, 1, 1)
    t = np.arange(NT).reshape(1, NT, 1)
    q = np.arange(PX).reshape(1, 1, PX)
    pix = PX * p + q                    # pixel index within a t-block
    w = np.empty((NP, NT, PX, 3), np.float32)
    w[..., 0] = 1.0
    w[..., 1] = pix % NY                               # j
    w[..., 2] = t * (NP * PX // NY) + pix // NY        # i
    return w


def _build():
    import base64
    import io

    import concourse.bacc as bacc
    import concourse.bass as bass
    import concourse.mybir as mybir
    import concourse.tile as tile

    F32 = mybir.dt.float32
    F32R = mybir.dt.float32r

    nc = bacc.Bacc("TRN2", target_bir_lowering=False)
    x_dram = nc.dram_tensor("x", [NF, NT, NP, NV], F32R, kind="ExternalInput")
    out_dram = nc.dram_tensor("out", [3, NF * NZ], F32, kind="ExternalOutput")

    # inline const weight table, declared float32r (bytes are plain fp32)
    W = _weights()
    mls = nc._tensor("w", list(W.shape), F32R, kind="Const", type="DRAM")
    buf = io.BytesIO()
    np.save(buf, W, allow_pickle=False)
    mls.file = "w.npy"
    mls.ant_data = base64.standard_b64encode(buf.getvalue()).decode()
    w_dram = bass.DRamTensorHandle("w", list(W.shape), F32R)

    with tile.TileContext(nc) as tc:
        with (
            tc.tile_pool(name="const", bufs=1) as const_pool,
            tc.tile_pool(name="xp", bufs=6) as xpool,
            tc.tile_pool(name="op", bufs=1) as opool,
            tc.tile_pool(name="ps", bufs=1, space=bass.MemorySpace.PSUM) as pspool,
        ):
            w_tile = const_pool.tile([NP, NT, PX, 3], F32R)
            nc.sync.dma_start(out=w_tile[:], in_=w_dram[:])

            acc = pspool.tile([3, NF, NZ], F32)

            for t in range(NT):
                xt = xpool.tile([NP, NF, PX, NZ], F32R)  # [p, f, q, z]
                # one DMA per t: per partition, 4 chunks of 1 KiB (one per f)
                nc.sync.dma_start(
                    out=xt[:],
                    in_=x_dram[:, t, :, :].rearrange("f p v -> p f v"),
                )
                for q in range(PX):
                    nc.tensor.matmul(
                        acc[:],
                        lhsT=w_tile[:, t, q, :],
                        rhs=xt[:, :, q, :],
                        start=(t == 0 and q == 0),
                        stop=(t == NT - 1 and q == PX - 1),
                    )

            res = opool.tile([3, NF * NZ], F32)
            nc.vector.tensor_copy(out=res[:], in_=acc[:].rearrange("c f z -> c (f z)"))
            nc.sync.dma_start(out=out_dram[:], in_=res[:])

    nc.compile()
    return nc


def _get_nc():
    if "nc" not in _CACHE:
        _CACHE["nc"] = _build()
    return _CACHE["nc"]


def kernel(x: np.ndarray) -> np.ndarray:
    from concourse.bass_utils import run_bass_kernel_spmd

    x = np.asarray(x)
    assert x.shape == (NB, NF, NX, NY, NZ), x.shape
    in_dtype = x.dtype
    x = np.ascontiguousarray(x, dtype=np.float32)

    nc = _get_nc()
    in_maps = [{"x": x[b].reshape(NF, NT, NP, NV)} for b in range(NB)]
    results = run_bass_kernel_spmd(nc, in_maps, core_ids=list(range(NB))).results

    out = np.empty((NB, NF, NZ), np.complex64)
    for b in range(NB):
        sums = np.asarray(results[b]["out"]).reshape(3, NF, NZ)
        mass = sums[0]
        out[b] = (sums[1] / mass + 1j * (sums[2] / mass)).astype(np.complex64)
    del in_dtype
    return out


# revision 14
# speedup vs baseline: 1.1420x; 1.1420x over previous
"""Trainium2 Bass kernel for CenterOfMass2DExtractor.

Full input x: (8, 4, 256, 256, 64) float32.  Output: (8, 4, 64) complex64
  mass[b,f,z]   = sum_{i,j} x[b,f,i,j,z]
  real[b,f,z]   = sum_{i,j} j * x / mass      (j = column index)
  imag[b,f,z]   = sum_{i,j} i * x / mass      (i = row index)

Sharding: pure data parallel over the batch dim -> 1 batch per NeuronCore
(8 cores), 64 MiB each, no communication.

Per-core kernel: view the shard as (f=4, t=NT, p=128, v=PX*64) where a
t-block covers 128*PX pixels (PX/2 image rows), partition p holds PX
consecutive pixels q=0..PX-1 (v = q*64 + z).  For each t: one PX*128 KiB
DMA (all 4 f), then PX matmuls (one per q) with a 3-column stationary
weight
  w[p, :] = [1, j(p,q), i(t,p,q)]
and moving operand (p, f, z) = 256 columns in float32r (full-rate fp32
on the PE), accumulating [mass, sum j*x, sum i*x] into a single
(3, 4, 64) PSUM tile across all 512 matmuls.  The tiny (3, 256) result
is copied to SBUF and DMA'd out; the divide by mass and the complex
assembly happen on host.

Hand-rolled raw-Bass engine programs (no TileContext): SP streams the x
DMAs with BUFS-slot ping-pong semaphores, ACT loads the weight table,
PE consumes, DVE does the final PSUM->SBUF copy.  Measured ~195 us/core
vs the ~186 us per-core HBM roofline (64 MiB @ ~360 GB/s).
"""

import numpy as np

_CACHE: dict = {}

NB, NF, NX, NY, NZ = 8, 4, 256, 256, 64
PX = 16           # pixels per partition per t-block
NT = 512 // PX    # t-blocks per f (128*PX pixels each)
NP = 128          # partitions
NV = PX * NZ      # values per partition per t-block


def _weights() -> np.ndarray:
    """(p, t, q, c) weight table: c = [mass, j, i]."""
    p = np.arange(NP).reshape(NP, 1, 1)
    t = np.arange(NT).reshape(1, NT, 1)
    q = np.arange(PX).reshape(1, 1, PX)
    pix = PX * p + q                    # pixel index within a t-block
    w = np.empty((NP, NT, PX, 3), np.float32)
    w[..., 0] = 1.0
    w[..., 1] = pix % NY                               # j
    w[..., 2] = t * (NP * PX // NY) + pix // NY        # i
    return w


BUFS = 6          # x-tile double buffering depth


def _build():
    import base64
    import io

    import concourse.bass as bass
    import concourse.mybir as mybir

    F32 = mybir.dt.float32
    F32R = mybir.dt.float32r

    nc = bass.Bass(trn_type="TRN2")
    x_dram = nc.dram_tensor("x", [NF, NT, NP, NV], F32R, kind="ExternalInput")
    out_dram = nc.dram_tensor("out", [3, NF * NZ], F32, kind="ExternalOutput")

    # inline const weight table, declared float32r (bytes are plain fp32)
    W = _weights()
    mls = nc._tensor("w", list(W.shape), F32R, kind="Const", type="DRAM")
    buf = io.BytesIO()
    np.save(buf, W, allow_pickle=False)
    mls.file = "w.npy"
    mls.ant_data = base64.standard_b64encode(buf.getvalue()).decode()
    w_dram = bass.DRamTensorHandle("w", list(W.shape), F32R)

    w_sb = nc.alloc_sbuf_tensor("w_sb", [NP, NT, PX, 3], F32R)
    xt = nc.alloc_sbuf_tensor("xt", [NP, BUFS, NF, PX, NZ], F32R)
    res = nc.alloc_sbuf_tensor("res", [3, NF * NZ], F32)
    acc = nc.alloc_psum_tensor("acc", [3, NF, NZ], F32)

    w_sem = nc.alloc_semaphore("w_sem")
    d = [nc.alloc_semaphore(f"d_sem{i}") for i in range(BUFS)]
    pe_sem = nc.alloc_semaphore("pe_sem")
    v_sem = nc.alloc_semaphore("v_sem")
    o_sem = nc.alloc_semaphore("o_sem")

    with nc.Block(no_gpsimd_drain=True) as block:

        @block.scalar
        def _(scalar: bass.BassEngine):
            # weight table on the ACT HWDGE ring so it doesn't delay x DMAs
            scalar.dma_start(out=w_sb[:], in_=w_dram[:]).then_inc(w_sem, 16)

        @block.sync
        def _(sync: bass.BassEngine):
            for t in range(NT):
                if t >= BUFS:
                    # slot reuse: wait until PE finished block t-BUFS
                    sync.wait_ge(pe_sem, t - BUFS + 1)
                sync.dma_start(
                    out=xt[:, t % BUFS],
                    in_=x_dram[:, t, :, :].rearrange("f p v -> p f v"),
                ).then_inc(d[t % BUFS], 16)
            sync.wait_ge(v_sem, 1)
            sync.dma_start(out=out_dram[:], in_=res[:]).then_inc(o_sem, 16)
            sync.wait_ge(o_sem, 16)

        @block.tensor
        def _(tensor: bass.BassEngine):
            tensor.wait_ge(w_sem, 16)
            for t in range(NT):
                tensor.wait_ge(d[t % BUFS], 16 * (t // BUFS + 1))
                for q in range(PX):
                    mm = tensor.matmul(
                        acc[:],
                        lhsT=w_sb[:, t, q, :],
                        rhs=xt[:, t % BUFS, :, q, :],
                        start=(t == 0 and q == 0),
                        stop=(t == NT - 1 and q == PX - 1),
                    )
                    if q == PX - 1:
                        mm.then_inc(pe_sem, 1)

        @block.vector
        def _(vector: bass.BassEngine):
            vector.wait_ge(pe_sem, NT)
            vector.tensor_copy(
                out=res[:], in_=acc[:].rearrange("c f z -> c (f z)")
            ).then_inc(v_sem, 1)

    return nc


def _get_nc():
    if "nc" not in _CACHE:
        _CACHE["nc"] = _build()
    return _CACHE["nc"]


def kernel(x: np.ndarray) -> np.ndarray:
    from concourse.bass_utils import run_bass_kernel_spmd

    x = np.ascontiguousarray(np.asarray(x), dtype=np.float32)
    assert x.shape == (NB, NF, NX, NY, NZ), x.shape

    nc = _get_nc()
    in_maps = [{"x": x[b].reshape(NF, NT, NP, NV)} for b in range(NB)]
    results = run_bass_kernel_spmd(nc, in_maps, core_ids=list(range(NB))).results

    out = np.empty((NB, NF, NZ), np.complex64)
    for b in range(NB):
        sums = np.asarray(results[b]["out"]).reshape(3, NF, NZ)
        mass = sums[0]
        out[b] = (sums[1] / mass + 1j * (sums[2] / mass)).astype(np.complex64)
    return out
